# revision 1
# baseline (speedup 1.0000x reference)
"""Causal single-head attention (Q==K source bug faithful) on 8 TRN2 NeuronCores.

Problem: x [4, 4096, 1024], Wk/Wv [1024, 64];
  k = q = x@Wk; scores = q k^T / 8, causal softmax, out = weights @ (x@Wv).

v5 strategy — balanced key-split with HOST-side softmax combine:
  - 8 cores = 4 batches x 2 "parities". Both cores of a batch project
    the full K (Q==K needs all queries anyway), but each core runs the
    attention for only HALF the key blocks: within each 1024-row key
    chunk j, core parity r owns the four 128-row key blocks at
    positions (j+r)%2 + {0,2,4,6}. This splits the causal triangle
    EXACTLY in half with a uniform 40-slot program per core (24 full
    off-diagonal slots + 16 trimmed diagonal slots) — no wasted
    compute, no cross-core traffic.
  - Each core emits per-chunk PARTIAL softmax accumulators
    [65, 1024] = (V|1)^T @ exp(S^T): rows 0..64 partial numerator^T,
    row 64 partial denominator, DMA'd straight from PSUM. The host
    adds the two cores' partials and divides — no epilogue transposes,
    no reciprocal, ~1us tail. (No max-subtraction needed: scores <=
    ~16, so partial sums stay well inside f32.)
  - Per-core data, uniform program: the host packs that core's own
    key blocks as x^T panels 0..3 and the peer's as panels 4..7, so
    all slot addressing (lhsT = own K^T block, Q = [own half | peer
    half] of a chunk) is core-independent; causal masks for the
    diagonal slots are a host-built table indexed by (chunk parity,
    block index).
  - Engine use: scores^T [keys, queries] on TensorE (bf16), exp on
    ScalarE (scale=1/8 fused) from PSUM, P@V accumulate on TensorE
    into the partial PSUM; software-pipelined one slot ahead so
    TensorE never waits on ScalarE; all panel DMAs issued upfront
    (panel-major contiguous layout, bf16).
"""
import numpy as np
import ml_dtypes

import concourse.bass as bass
import concourse.mybir as mybir
from concourse import bacc, tile
from concourse.bass_utils import run_bass_kernel_spmd

F32 = mybir.dt.float32
BF16 = mybir.dt.bfloat16
EXP = mybir.ActivationFunctionType.Exp

B, T, C, H = 4, 4096, 1024, 64
NCHI = C // 128          # 8 contraction blocks
NPAN = 8                 # panels (0..3 own, 4..7 peer), 512 rows each
PAN = 512
CHUNK = 1024             # queries per chunk
NCK = T // CHUNK         # 4 chunks


def build_nc():
    nc = bacc.Bacc("TRN2", target_bir_lowering=False, debug=False, num_devices=8)

    xt_d = nc.declare_dram_parameter("xt", [NPAN, 128, NCHI, PAN], BF16, isOutput=False)
    wkv_d = nc.declare_dram_parameter("wkv", [128, NCHI, 128], BF16, isOutput=False)
    gm_d = nc.declare_dram_parameter("gm", [128, PAN], BF16, isOutput=False)
    mb_d = nc.declare_dram_parameter("mb", [128, 2, 128], BF16, isOutput=False)
    eyeb_d = nc.declare_dram_parameter("eyeb", [128, 65], BF16, isOutput=False)
    out_d = nc.declare_dram_parameter("out", [NCK, 65, 1024], F32, isOutput=True)

    with tile.TileContext(nc) as tc:
        with (
            tc.tile_pool(name="const", bufs=1) as const,
            tc.tile_pool(name="xt", bufs=NPAN) as xtp,
            tc.tile_pool(name="kv", bufs=3) as kvp,
            tc.tile_pool(name="pt", bufs=8) as ptp,
            tc.tile_pool(name="osb", bufs=2) as osbp,
            tc.tile_pool(name="psA", bufs=2, space="PSUM") as psA,
            tc.tile_pool(name="psO", bufs=2, space="PSUM") as psO,
        ):
            wkv = const.tile([128, NCHI, 128], BF16, tag="wkv")
            gm = const.tile([128, PAN], BF16, tag="gm")
            mb = const.tile([128, 2, 128], BF16, tag="mb")
            eyeb = const.tile([128, 65], BF16, tag="eyeb")
            kt = const.tile([64, T], BF16, tag="kt")     # [own 2048 | peer 2048]
            vaug = const.tile([128, 16, 65], BF16, tag="vaug")  # own V|1 per kb

            nc.gpsimd.dma_start(wkv[:], wkv_d[:])
            nc.sync.dma_start(gm[:], gm_d[:])
            nc.sync.dma_start(mb[:], mb_d[:])
            nc.sync.dma_start(eyeb[:], eyeb_d[:])
            # ones column of every V|1 block
            nc.vector.tensor_copy(
                vaug[:, :, 64:65],
                eyeb[:, 64:65].unsqueeze(1).broadcast_to([128, 16, 1]),
            )

            for _ in range(2):
                z = psA.tile([128, 1024], F32, tag="ps", name="z")
                nc.scalar.memzero(z[:])

            # All panel DMAs upfront: own panels on one queue, peer on the
            # other, so panels 0 and 4 land first.
            xts = []
            for p in range(NPAN):
                xt = xtp.tile([128, NCHI, PAN], BF16, tag="xt")
                (nc.gpsimd if p < 4 else nc.sync).dma_start(xt[:], xt_d[p])
                xts.append(xt)

            def proj_panel(p):
                """Project K^T (and V^T for own panels) of panel p."""
                xt = xts[p]
                pj = psA.tile([128, 1024], F32, tag="ps")
                kv_ps = pj[:, 0:PAN]
                if p >= 4:  # peer panel: only K^T needed (queries), m=64
                    for ci in range(NCHI):
                        nc.tensor.matmul(
                            pj[0:64, 0:PAN], wkv[:, ci, 0:64], xt[:, ci, :],
                            start=(ci == 0), stop=(ci == NCHI - 1),
                        )
                    nc.vector.tensor_copy(kt[:, p * PAN:(p + 1) * PAN], pj[0:64, 0:PAN])
                    return
                for ci in range(NCHI):
                    nc.tensor.matmul(
                        kv_ps[:], wkv[:, ci, :], xt[:, ci, :],
                        start=(ci == 0), stop=(ci == NCHI - 1),
                    )
                kvsb = kvp.tile([128, PAN], BF16, tag="kv")
                nc.vector.tensor_copy(kvsb[:], kv_ps[:])
                nc.vector.tensor_copy(kt[:, p * PAN:(p + 1) * PAN], kvsb[0:64, :])
                v_ps = pj[:, PAN:PAN + 128].bitcast(BF16).rearrange(
                    "p (a b) -> p a b", a=4)
                for tb in range(4):
                    nc.tensor.transpose(
                        v_ps[:, tb, :], kvsb[64:128, tb * 128:(tb + 1) * 128],
                        eyeb[64:128, 0:64],
                    )
                nc.vector.tensor_copy(vaug[:, 4 * p:4 * p + 4, 0:64], v_ps[:])

            ot = {}  # live chunk accumulators [65, 1024]

            def scores_exp(j, lkb, i=None):
                """Slot front half. lkb = own local key block (0..15).
                i = None: off-diagonal (full). i = 0..3: diagonal block
                index; both halves trimmed to suffix [128i:]; the exp
                spans [128i:1024] (the stale gap [512:512+128i] is
                pre-zeroed/finite and never read by PV); gm masks the
                own triangle, mb zeroes the first peer block when this
                chunk parity makes it non-causal."""
                c0 = 0 if i is None else 128 * i
                s_ps = psA.tile([128, 1024], F32, tag="ps")
                lhsT = kt[:, lkb * 128:(lkb + 1) * 128]
                nc.tensor.matmul(
                    s_ps[:, c0:512],
                    lhsT, kt[:, j * PAN + c0:(j + 1) * PAN],
                    start=True, stop=True,
                )
                nc.tensor.matmul(
                    s_ps[:, 512 + c0:1024],
                    lhsT, kt[:, 2048 + j * PAN + c0:2048 + (j + 1) * PAN],
                    start=True, stop=True,
                )
                pt = ptp.tile([128, 1024], BF16, tag="pt")
                nc.scalar.activation(pt[:, c0:1024], s_ps[:, c0:1024], EXP, scale=0.125)
                if i is not None:
                    nc.vector.tensor_mul(
                        pt[:, c0:512], pt[:, c0:512], gm[:, 0:512 - c0]
                    )
                    nc.vector.tensor_mul(
                        pt[:, 512 + c0:512 + c0 + 128],
                        pt[:, 512 + c0:512 + c0 + 128], mb[:, j % 2, :]
                    )
                return pt

            def pv(j, lkb, pt, i=None, first=False, last=False):
                c0 = 0 if i is None else 128 * i
                if first:
                    ot[j] = psO.tile([65, 1024], F32, tag="ot", name="ot")
                acc = ot[j]
                nc.tensor.matmul(
                    acc[:, c0:512], vaug[:, lkb, :], pt[:, c0:512],
                    start=first, stop=last,
                )
                nc.tensor.matmul(
                    acc[:, 512 + c0:1024], vaug[:, lkb, :], pt[:, 512 + c0:1024],
                    start=first, stop=last,
                )

            # ---- schedule: per chunk j: off-diagonal kbs then diagonal ----
            def chunk_slots(j):
                s = []
                for lkb in range(4 * j):
                    s.append(("slot", dict(j=j, lkb=lkb, first=(lkb == 0))))
                for i in range(4):
                    s.append(("slot", dict(
                        j=j, lkb=4 * j + i, i=i,
                        first=(j == 0 and i == 0), last=(i == 3))))
                return s

            actions = [("proj", 0), ("proj", 4)]
            actions += chunk_slots(0) + [("out", 0)]
            actions += [("proj", 1), ("proj", 5)]
            actions += chunk_slots(1) + [("out", 1)]
            actions += [("proj", 2), ("proj", 6), ("proj", 3), ("proj", 7)]
            c2, c3 = chunk_slots(2), chunk_slots(3)
            mix = []
            while c2 or c3:  # interleave 2:3 to even out the tail
                if c2:
                    mix.append(c2.pop(0))
                if c3:
                    mix.append(c3.pop(0))
                if c3:
                    mix.append(c3.pop(0))
            for a in mix:
                actions.append(a)
                if a[1].get("last") and a[1]["j"] == 2:
                    actions.append(("out", 2))
            actions.append(("out", 3))

            pending = None

            def flush():
                nonlocal pending
                if pending is not None:
                    a, pt = pending
                    pv(a["j"], a["lkb"], pt, i=a.get("i"),
                       first=a.get("first", False), last=a.get("last", False))
                    pending = None

            for kind, arg in actions:
                if kind == "proj":
                    proj_panel(arg)
                elif kind == "slot":
                    pt = scores_exp(arg["j"], arg["lkb"], i=arg.get("i"))
                    flush()
                    pending = (arg, pt)
                else:  # out: partials via an SBUF bounce
                    flush()
                    osb = osbp.tile([65, 1024], F32, tag="osb")
                    nc.vector.tensor_copy(osb[:], ot[arg][:])
                    nc.sync.dma_start(out_d.ap()[arg], osb[:])

    nc.compile()
    return nc


def _own_blocks(j, r):
    """Global 128-row key-block positions (within chunk j) owned by
    parity r, in local order."""
    q = (j + r) % 2
    return [q + 2 * i for i in range(4)]


def make_inputs(x, Wk, Wv):
    """Build the 8 per-core input maps (pure layout work)."""
    bf16 = ml_dtypes.bfloat16
    wkv = np.concatenate([Wk, Wv], axis=1)            # [1024, 128]
    wkv_t = np.ascontiguousarray(
        wkv.reshape(NCHI, 128, 128).transpose(1, 0, 2)
    ).astype(bf16)  # [cp, chi, m]

    eyeb = np.zeros((128, 65), dtype=np.float32)
    eyeb[64:128, 0:64] = np.eye(64)
    eyeb[:, 64] = 1.0
    eyeb = eyeb.astype(bf16)

    pp = np.arange(128)[:, None]                      # key row within block
    cc = np.arange(1024)[None, :]                     # query column

    in_maps = []
    for c in range(8):
        b, r = c % 4, c // 4

        # x^T panels: own rows (panels 0..3) then peer rows (4..7)
        xT = np.ascontiguousarray(x[b].T)             # [1024, 4096]
        xr = xT.reshape(NCHI, 128, T)                 # [chi, cp, t]
        xt = np.empty((NPAN, 128, NCHI, PAN), dtype=bf16)
        for p in range(NPAN):
            j, rr = (p, r) if p < 4 else (p - 4, 1 - r)
            rows = np.concatenate([
                np.arange(j * CHUNK + m * 128, j * CHUNK + m * 128 + 128)
                for m in _own_blocks(j, rr)
            ])
            xt[p] = xr[:, :, rows].transpose(1, 0, 2)

        # Own-half triangle mask (block 0 triu, rest ones) and the
        # per-chunk-parity first-peer-block mask (zero iff own parity 1).
        gmm = np.ones((128, PAN), dtype=np.float32)
        gmm[:, 0:128] = (cc[:, 0:128] >= pp).astype(np.float32)
        mbm = np.empty((128, 2, 128), dtype=np.float32)
        for jp in range(2):
            mbm[:, jp, :] = 0.0 if (jp + r) % 2 == 1 else 1.0

        in_maps.append({"xt": xt, "wkv": wkv_t, "gm": gmm.astype(bf16),
                        "mb": mbm.astype(bf16), "eyeb": eyeb})
    return in_maps


_NC = None


def get_nc():
    global _NC
    if _NC is None:
        _NC = build_nc()
    return _NC


def kernel(x, Wk, Wv):
    x = np.asarray(x, dtype=np.float32)
    Wk = np.asarray(Wk, dtype=np.float32)
    Wv = np.asarray(Wv, dtype=np.float32)
    nc = get_nc()
    in_maps = make_inputs(x, Wk, Wv)
    res = run_bass_kernel_spmd(nc, in_maps, list(range(8)))

    out = np.empty((B, T, H), dtype=np.float32)
    for b in range(4):
        p0 = res.results[b]["out"].astype(np.float64)      # parity 0
        p1 = res.results[b + 4]["out"].astype(np.float64)  # parity 1
        for j in range(NCK):
            tot = np.zeros((65, 1024), dtype=np.float64)
            for r, part in ((0, p0[j]), (1, p1[j])):
                blocks = _own_blocks(j, r) + _own_blocks(j, 1 - r)
                glob = np.empty((65, 1024), dtype=np.float64)
                for k, m in enumerate(blocks):
                    glob[:, m * 128:(m + 1) * 128] = part[:, k * 128:(k + 1) * 128]
                tot += glob
            out[b, j * CHUNK:(j + 1) * CHUNK] = (tot[0:64] / tot[64]).T
    return out

